# revision 1
# baseline (speedup 1.0000x reference)
"""MoE MLP (sigmoid router, top-2, relu^2 experts) on 8 Trainium2 cores.

Sharding: pure token/data parallel. Each of the 8 cores takes T/8 = 512
tokens and computes the full dense MoE for them (router fp32; expert
matmuls in fp32r). No collectives; the host concatenates the 8 output
shards.

Per-core layout ("orientation 1" — transposed activations):
  xT   [D=1024 on partitions (8 tiles of 128), T=512 free]
  h_e^T = w1_e^T-slices (lhsT, native DRAM layout) @ xT   -> PSUM
  a = relu(h)^2 * cw_bcast (cw broadcast across partitions)
  y^T[dc] += w2_e-slices (lhsT, native DRAM layout) @ a   -> PSUM, 2 D-halves
  acc[dc] (SBUF fp32) accumulates y over experts; final PE transpose back
  to token-major and DMA out.

Everything is hardcoded for the fixed problem shapes:
  x [2,2048,1024] f32, router_w [8,1024], w1 [1024,8192], w2 [8192,1024].
"""

import os

import numpy as np

import concourse.bacc as bacc
import concourse.bass as bass
import concourse.mybir as mybir
import concourse.tile as tile
from concourse.bass_utils import run_bass_kernel_spmd
from concourse.masks import make_identity

N_CORES = 8
B, S, D = 2, 2048, 1024
T = B * S  # 4096
TS = T // N_CORES  # 512 tokens per core
E = 8
W = 1024  # width per expert
NDC = D // 128  # 8 D-chunks
NWC = W // 128  # 8 W-chunks per expert
NTT = TS // 128  # 4 token tiles
DH = 2  # D halves for down-proj PSUM rotation
NDH = NDC // DH  # 4 D-chunks per half

F32 = mybir.dt.float32
# fp32r: single-pass reduced-precision fp32 matmul (4x faster than fp32).
# Set MOE_MM_DTYPE=f32 to fall back to full fp32 matmuls.
DT_MM = F32 if os.environ.get("MOE_MM_DTYPE") == "f32" else mybir.dt.float32r

AF = mybir.ActivationFunctionType
ALU = mybir.AluOpType


def build_nc():
    nc = bacc.Bacc(
        "TRN2", target_bir_lowering=False, debug=False, num_devices=N_CORES
    )
    x = nc.dram_tensor("x", [TS, D], F32, kind="ExternalInput")
    rw = nc.dram_tensor("router_w", [E, D], F32, kind="ExternalInput")
    w1 = nc.dram_tensor("w1", [D, E * W], DT_MM, kind="ExternalInput")
    w2 = nc.dram_tensor("w2", [E * W, D], DT_MM, kind="ExternalInput")
    out = nc.dram_tensor("out", [TS, D], F32, kind="ExternalOutput")
    dbg = {}
    if os.environ.get("MOE_DEBUG"):
        dbg["logits"] = nc.dram_tensor("dbg_logits", [128, NTT * E], F32, kind="ExternalOutput")
        dbg["cw"] = nc.dram_tensor("dbg_cw", [128, NTT * E], F32, kind="ExternalOutput")
        dbg["cwb0"] = nc.dram_tensor("dbg_cwb0", [128, TS], F32, kind="ExternalOutput")
        dbg["cwb5"] = nc.dram_tensor("dbg_cwb5", [128, TS], F32, kind="ExternalOutput")
        dbg["xT0"] = nc.dram_tensor("dbg_xT0", [128, TS], F32, kind="ExternalOutput")
        dbg["a00"] = nc.dram_tensor("dbg_a00", [128, TS], F32, kind="ExternalOutput")
        dbg["acc0"] = nc.dram_tensor("dbg_acc0", [128, TS], F32, kind="ExternalOutput")
        dbg["rwT"] = nc.dram_tensor("dbg_rwT", [128, E * NDC], F32, kind="ExternalOutput")
        dbg["rawlogits"] = nc.dram_tensor("dbg_rawlogits", [128, NTT * E], F32, kind="ExternalOutput")

    with tile.TileContext(nc) as tc:
        with (
            tc.tile_pool(name="persist", bufs=1) as persist,
            tc.tile_pool(name="xs", bufs=2) as xsp,
            tc.tile_pool(name="w1p", bufs=12) as w1p,
            tc.tile_pool(name="w2p", bufs=12) as w2p,
            tc.tile_pool(name="ap", bufs=10) as ap_,
            tc.tile_pool(name="relp", bufs=3) as relp,
            tc.tile_pool(name="cwbp", bufs=2) as cwbp,
            tc.tile_pool(name="outp", bufs=1) as outp,
            tc.tile_pool(name="psA", bufs=4, space="PSUM") as psA,
            tc.tile_pool(name="psY", bufs=4, space="PSUM") as psY,
        ):
            ident = persist.tile([128, 128], F32, tag="ident")
            make_identity(nc, ident[:])

            # ---------------- phase 0: load x, transpose, router ----------
            rw_t = persist.tile([E, D], F32, tag="rw")
            nc.sync.dma_start(rw_t[:], rw[:])
            xT = [
                persist.tile([128, TS], DT_MM, tag=f"xT{dc}", name=f"xT{dc}")
                for dc in range(NDC)
            ]
            # rwT[:, dc*8:(dc+1)*8] = router_w[:, dc-chunk].T  (f32: the
            # router must be exact — f32r logits noise flips top-2 near-ties)
            rwT = persist.tile([128, E * NDC], F32, tag="rwT")
            for dc in range(NDC):
                p = psA.tile([128, E], F32, tag="h")
                nc.tensor.transpose(
                    p[:], rw_t[0:E, dc * 128 : (dc + 1) * 128], ident[0:E, 0:E]
                )
                nc.vector.tensor_copy(rwT[:, dc * E : (dc + 1) * E], p[:])

            # logits PSUM tile: [:, tt*8:(tt+1)*8] holds logits of token tile tt
            logits = psY.tile([128, NTT * E], F32, tag="y")
            for tt in range(NTT):
                t = xsp.tile([128, D], F32, tag="xs", name=f"xs{tt}")
                nc.sync.dma_start(t[:], x[tt * 128 : (tt + 1) * 128, :])
                for dc in range(NDC):
                    p = psA.tile([128, 128], F32, tag="h")
                    nc.tensor.transpose(
                        p[:], t[:, dc * 128 : (dc + 1) * 128], ident[:]
                    )
                    # f32 scratch block feeds the exact-f32 router matmul;
                    # xT keeps the f32r copy for the expert matmuls
                    xtb = xsp.tile([128, 128], F32, tag="xtb", name=f"xtb{tt}_{dc}")
                    nc.vector.tensor_copy(xtb[:], p[:])
                    nc.vector.tensor_copy(
                        xT[dc][:, tt * 128 : (tt + 1) * 128], p[:]
                    )
                    # start=True clears has_written for the whole PSUM bank,
                    # so only the very first MM into the bank may set it;
                    # later groups overwrite-by-has_written=0 instead.
                    nc.tensor.matmul(
                        logits[:, tt * E : (tt + 1) * E],
                        xtb[:],
                        rwT[:, dc * E : (dc + 1) * E],
                        start=(dc == 0 and tt == 0),
                        stop=(dc == NDC - 1),
                    )

            # router probs + top-2 normalized combine weights, per token tile
            pr = persist.tile([128, NTT * E], F32, tag="pr")
            cw = persist.tile([128, NTT * E], F32, tag="cw")
            mstat = persist.tile([128, NTT * 4], F32, tag="mstat")
            tmp = persist.tile([128, NTT * E], F32, tag="cwtmp")
            for tt in range(NTT):
                prt = pr[:, tt * E : (tt + 1) * E]
                tmpt = tmp[:, tt * E : (tt + 1) * E]
                m1 = mstat[:, tt * 4 : tt * 4 + 1]
                m2 = mstat[:, tt * 4 + 1 : tt * 4 + 2]
                den = mstat[:, tt * 4 + 2 : tt * 4 + 3]
                rden = mstat[:, tt * 4 + 3 : tt * 4 + 4]
                cwt = cw[:, tt * E : (tt + 1) * E]
                nc.scalar.activation(prt, logits[:, tt * E : (tt + 1) * E], AF.Sigmoid)
                nc.vector.reduce_max(m1, prt, axis=mybir.AxisListType.X)
                # mask out the max, re-max to get 2nd largest (probs > 0)
                nc.vector.tensor_scalar(tmpt, prt, m1, None, op0=ALU.is_lt)
                nc.vector.tensor_mul(tmpt, tmpt, prt)
                nc.vector.reduce_max(m2, tmpt, axis=mybir.AxisListType.X)
                nc.vector.tensor_add(den, m1, m2)
                nc.vector.tensor_scalar(den, den, 1e-20, None, op0=ALU.add)
                nc.vector.reciprocal(rden, den)
                # cw = pr * (pr >= m2) * rden
                nc.vector.tensor_scalar(cwt, prt, m2, None, op0=ALU.is_ge)
                nc.vector.tensor_mul(cwt, cwt, prt)
                nc.vector.tensor_scalar(cwt, cwt, rden, None, op0=ALU.mult)

            if dbg:
                nc.sync.dma_start(dbg["logits"][:], pr[:])
                nc.sync.dma_start(dbg["rwT"][:], rwT[:])
                rawl = persist.tile([128, NTT * E], F32, tag="rawl")
                nc.vector.tensor_copy(rawl[:], logits[:])
                nc.sync.dma_start(dbg["rawlogits"][:], rawl[:])

            # cwT[e, t] then broadcast to cwb[e] [128, TS]
            cwT = persist.tile([E, TS], F32, tag="cwT")
            for tt in range(NTT):
                p = psA.tile([E, 128], F32, tag="h")
                nc.tensor.transpose(p[:], cw[:, tt * E : (tt + 1) * E], ident[:])
                nc.vector.tensor_copy(cwT[:, tt * 128 : (tt + 1) * 128], p[:])
            def make_cwb(e):
                # partition_broadcast needs its source at partition 0
                r = xsp.tile([1, TS], F32, tag="cwr", name=f"cwr{e}")
                nc.sync.dma_start(r[:], cwT[e : e + 1, :])
                t = cwbp.tile([128, TS], F32, tag="cwb", name=f"cwb{e}")
                nc.gpsimd.partition_broadcast(t[:], r[:])
                return t

            if dbg:
                nc.sync.dma_start(dbg["cw"][:], cw[:])
                nc.gpsimd.dma_start(dbg["xT0"][:], xT[0][:])

            # ---------------- expert loop --------------------------------
            outm = [
                outp.tile([128, D], F32, tag=f"outm{tt}", name=f"outm{tt}")
                for tt in range(NTT)
            ]
            acc = [
                persist.tile([128, TS], F32, tag=f"acc{dc}", name=f"acc{dc}")
                for dc in range(NDC)
            ]
            for e in range(E):
                cwb_e = make_cwb(e)
                # up-proj, dcc-major in 2 groups of 4 wc. w1 streams as
                # [128, 512] tiles per (dcc, group) so PE consumes the w1
                # stream tile-by-tile instead of stalling on a 4 MB slab.
                a_tiles = [None] * NWC
                for g in range(2):
                    w1f = []
                    for dcc in range(NDC):
                        t = w1p.tile([128, W // 2], DT_MM, tag="w1", name=f"w1_{e}_{g}_{dcc}")
                        nc.sync.dma_start(
                            t[:],
                            w1[
                                dcc * 128 : (dcc + 1) * 128,
                                e * W + g * (W // 2) : e * W + (g + 1) * (W // 2),
                            ],
                        )
                        w1f.append(t)
                    hs = [
                        psA.tile([128, TS], F32, tag="h", name=f"h{e}_{g}_{k}")
                        for k in range(4)
                    ]
                    for dcc in range(NDC):
                        for k in range(4):
                            nc.tensor.matmul(
                                hs[k][:],
                                w1f[dcc][:, k * 128 : (k + 1) * 128],
                                xT[dcc][:],
                                start=(dcc == 0),
                                stop=(dcc == NDC - 1),
                            )
                    for k in range(4):
                        wc = g * 4 + k
                        rel = relp.tile([128, TS], F32, tag="rel")
                        nc.scalar.activation(rel[:], hs[k][:], AF.Relu)
                        a_t = ap_.tile([128, TS], DT_MM, tag="a")
                        nc.vector.tensor_mul(a_t[:], rel[:], rel[:])
                        nc.vector.tensor_mul(a_t[:], a_t[:], cwb_e[:])
                        a_tiles[wc] = a_t
                        if dbg and e == 0 and wc == 0:
                            nc.gpsimd.dma_start(dbg["a00"][:], a_t[:])

                for half in range(DH):
                    ys = [
                        psY.tile([128, TS], F32, tag="y", name=f"y{e}_{half}_{j}")
                        for j in range(NDH)
                    ]
                    for wc in range(NWC):
                        # stream w2_e chunk: [128 (W-chunk), 512 (D-half)]
                        w2t = w2p.tile([128, D // DH], DT_MM, tag="w2")
                        nc.sync.dma_start(
                            w2t[:],
                            w2[
                                e * W + wc * 128 : e * W + (wc + 1) * 128,
                                half * (D // DH) : (half + 1) * (D // DH),
                            ],
                        )
                        for j in range(NDH):
                            nc.tensor.matmul(
                                ys[j][:],
                                w2t[:, j * 128 : (j + 1) * 128],
                                a_tiles[wc][:],
                                start=(wc == 0),
                                stop=(wc == NWC - 1),
                            )
                    for j in range(NDH):
                        dc = half * NDH + j
                        if e == 0:
                            nc.vector.tensor_copy(acc[dc][:], ys[j][:])
                        else:
                            nc.vector.tensor_add(acc[dc][:], acc[dc][:], ys[j][:])
                    if e == E - 1:
                        # final expert: transpose this half's finished acc
                        # back to token-major and store it now, overlapping
                        # the other down-pass (PE is in-order; emitting later
                        # would serialize all output work after the last MM)
                        lo, hi = half * NDH * 128, (half + 1) * NDH * 128
                        for dc in range(half * NDH, (half + 1) * NDH):
                            for tt in range(NTT):
                                p = psA.tile([128, 128], F32, tag="h")
                                nc.tensor.transpose(
                                    p[:],
                                    acc[dc][:, tt * 128 : (tt + 1) * 128],
                                    ident[:],
                                )
                                nc.vector.tensor_copy(
                                    outm[tt][:, dc * 128 : (dc + 1) * 128], p[:]
                                )
                        for tt in range(NTT):
                            nc.sync.dma_start(
                                out[tt * 128 : (tt + 1) * 128, lo:hi],
                                outm[tt][:, lo:hi],
                            )

            if dbg:
                nc.sync.dma_start(dbg["acc0"][:], acc[0][:])

    nc.compile()
    return nc


_NC_CACHE = None


def get_nc():
    global _NC_CACHE
    if _NC_CACHE is None:
        _NC_CACHE = build_nc()
    return _NC_CACHE


def make_in_maps(x, router_w, w1, w2):
    xf = np.ascontiguousarray(np.asarray(x, dtype=np.float32).reshape(T, D))
    router_w = np.ascontiguousarray(np.asarray(router_w, dtype=np.float32))
    w1 = np.ascontiguousarray(np.asarray(w1, dtype=np.float32))
    w2 = np.ascontiguousarray(np.asarray(w2, dtype=np.float32))
    return [
        {
            "x": xf[c * TS : (c + 1) * TS],
            "router_w": router_w,
            "w1": w1,
            "w2": w2,
        }
        for c in range(N_CORES)
    ]


def kernel(x, router_w, w1, w2):
    nc = get_nc()
    in_maps = make_in_maps(x, router_w, w1, w2)
    res = run_bass_kernel_spmd(nc, in_maps, list(range(N_CORES)))
    out = np.concatenate([res.results[c]["out"] for c in range(N_CORES)], axis=0)
    return out.reshape(B, S, D).astype(np.float32)

